# revision 5
# baseline (speedup 1.0000x reference)
"""Masked (expander) linear layer on 8 Trainium2 NeuronCores.

Computes out = x @ (W * M)^T for
  x: [16384, 2048] f32, W: [2048, 2048] f32, M: [2048, 2048] int32 (0/1)

Sharding: pure data-parallel over rows of x. Each of the 8 cores gets 2048
rows of x plus a replicated (transposed) copy of W and M, computes its
[2048, 2048] output shard entirely locally (mask-multiply on DVE, matmul on
PE), and the host concatenates shards. No collectives.

Device-side design (v4, bf16):
 - All matmuls run in bf16 (1 PE cycle/row -- same peak as f32r -- but
   LDWEIGHTS gets Fast-Weight-Load, ~100ns vs ~227ns for fp32, so weight
   loads hide behind the 512-cycle moving stream; measured MATMUL spacing
   sits at the 216ns streaming floor). PSUM accumulates f32 over the full
   K=2048; outputs stored f32. Measured rel err ~2.2e-3.
 - Transport is bf16/int8 (host pre-packs; bit-identical to an on-device
   cast since mask is 0/1). All module arithmetic (mask multiply, matmul)
   stays on device.
 - W pieces DMA directly into the resident wm tiles (no staging pool);
   all 16 mask tiles are resident; the DVE mask-multiply runs in place.
   This removes every WAR dependency from the input stream.
 - One input ring: W pieces and x stream on the sync HWDGE ring,
   interleaved in deadline order (piece (nt0,q) alternating with x
   quarter q, then remaining W pieces, then block-1 x). FIFO order makes
   the block-1 x prefetch self-throttle behind the W stream instead of
   stealing ramp bandwidth. Masks ride the scalar ring first.
 - m-tiles are processed in blocks of 8 (nt-outer inside a block) so each
   W piece unlocks 32 matmuls (~7us) vs ~2us delivery. x is
   double-buffered across blocks. PSUM groups rotate over all 8 banks and
   evacuate immediately on close (copy alternates ScalarE/VectorE; store
   DMA alternates scalar/gpsimd rings, keeping sync clear for inputs).
 - A short warmup burst of junk matmuls (on a zeroed tile, PSUM bank 0)
   runs during the first DMA wait so the PE HAM clock-gate is already at
   full rate (2.4 GHz) when the real matmuls arrive.
"""

from contextlib import ExitStack

import numpy as np

import concourse.bacc as bacc
import concourse.bass as bass
import concourse.mybir as mybir
import concourse.tile as tile
from concourse.bass_utils import run_bass_kernel_spmd

N_CORES = 8
P = 128

FULL_N, FULL_OUT, FULL_IN = 16384, 2048, 2048


def build_nc(
    rows: int = FULL_N // N_CORES,
    in_dim: int = FULL_IN,
    out_dim: int = FULL_OUT,
    n_chunk: int = 512,
    m_block: int = 8,
    warmup_mms: int = 12,
):
    """Per-core Bass module: y[rows, out] = x @ (wt * m).

    DRAM layouts: wt/mk panel-major [NT, in_dim, n_chunk] (wt bf16, mk int8);
    x transposed bf16 [in_dim, rows]; y row-major f32 [rows, out_dim].
    """
    assert rows % P == 0 and in_dim % P == 0 and out_dim % n_chunk == 0
    KT = in_dim // P
    MT = rows // P
    NT = out_dim // n_chunk
    assert KT % 4 == 0 and MT % m_block == 0
    KQ = KT // 4
    NB = MT // m_block
    mw = m_block * P  # columns of x per block

    bf16 = mybir.dt.bfloat16

    nc = bacc.Bacc("TRN2", target_bir_lowering=False, debug=False)
    x = nc.dram_tensor("x", [in_dim, rows], bf16, kind="ExternalInput")
    wt = nc.dram_tensor("wt", [NT, in_dim, n_chunk], bf16, kind="ExternalInput")
    mk = nc.dram_tensor("mk", [NT, in_dim, n_chunk], mybir.dt.int8, kind="ExternalInput")
    y = nc.dram_tensor("y", [rows, out_dim], mybir.dt.float32, kind="ExternalOutput")

    # K-major DRAM views: [.., p, kt, ..]
    wt_v = wt[:, :, :].rearrange("t (kt p) n -> t p kt n", p=P)
    mk_v = mk[:, :, :].rearrange("t (kt p) n -> t p kt n", p=P)
    x_v = x[:, :].rearrange("(kt p) m -> p kt m", p=P)

    with ExitStack() as ctx:
        tc = ctx.enter_context(tile.TileContext(nc))
        wm_pool = ctx.enter_context(tc.tile_pool(name="wm", bufs=1))
        mk_pool = ctx.enter_context(tc.tile_pool(name="mk", bufs=1))
        xt_pool = ctx.enter_context(tc.tile_pool(name="xt", bufs=1))
        yo_pool = ctx.enter_context(tc.tile_pool(name="yo", bufs=4))
        wu_pool = ctx.enter_context(tc.tile_pool(name="wu", bufs=1))
        pm_pool = ctx.enter_context(tc.tile_pool(name="pm", bufs=1, space="PSUM"))

        # Resident masked weight: wm_t[nt][q] of shape [P, KQ, n_chunk]
        wm_t = [
            [
                wm_pool.tile([P, KQ, n_chunk], bf16, tag=f"wm{nt}_{q}", name=f"wm{nt}_{q}")
                for q in range(4)
            ]
            for nt in range(NT)
        ]
        # Resident masks, same panel shapes
        mk_t = [
            [
                mk_pool.tile(
                    [P, KQ, n_chunk], mybir.dt.int8, tag=f"mk{nt}_{q}", name=f"mk{nt}_{q}"
                )
                for q in range(4)
            ]
            for nt in range(NT)
        ]
        # x tiles: double-buffered per block parity: [set][q] -> [P, KQ, mw]
        xt_t = [
            [
                xt_pool.tile([P, KQ, mw], bf16, tag=f"xt{s}_{q}", name=f"xt{s}_{q}")
                for q in range(4)
            ]
            for s in range(2)
        ]

        # ---- PE warmup: junk matmuls on a zeroed tile into PSUM bank 0 ----
        # Keeps the PE busy during the first DMA wait so the HAM clock-gate
        # reaches 8/8 before the real matmul stream begins.
        if warmup_mms:
            wutile = wu_pool.tile([P, n_chunk], bf16, tag="wu", name="wu")
            nc.vector.memzero(wutile[:])
            pmw = pm_pool.tile([P, n_chunk], mybir.dt.float32, tag="pm0", name="pm_warm")
            for i in range(warmup_mms):
                nc.tensor.matmul(
                    pmw[:],
                    wutile[:, :P],
                    wutile[:],
                    start=(i == 0),
                    stop=(i == warmup_mms - 1),
                )

        def load_w_piece(nt, q):
            ksl = slice(q * KQ, (q + 1) * KQ)
            nc.sync.dma_start(out=wm_t[nt][q][:], in_=wt_v[nt, :, ksl, :])

        def mul_piece(nt, q):
            for k in range(KQ):
                nc.vector.tensor_mul(
                    wm_t[nt][q][:, k, :], wm_t[nt][q][:, k, :], mk_t[nt][q][:, k, :]
                )

        def load_x_piece(b, q, half=None):
            ksl = slice(q * KQ, (q + 1) * KQ)
            hw = mw // 2
            xt = xt_t[b % 2][q]
            halves = range(2) if half is None else [half]
            for h in halves:
                nc.sync.dma_start(
                    out=xt[:, :, h * hw : (h + 1) * hw],
                    in_=x_v[:, ksl, b * mw + h * hw : b * mw + (h + 1) * hw],
                )

        # ---- prep: masks on the scalar ring; W+x interleaved on sync ----
        for nt in range(NT):
            for q in range(4):
                nc.scalar.dma_start(out=mk_t[nt][q][:], in_=mk_v[nt, :, q * KQ : (q + 1) * KQ, :])
        # deadline order: nt0 pieces and block-0 x quarters interleave
        for q in range(4):
            load_w_piece(0, q)
            mul_piece(0, q)
            load_x_piece(0, q)
        for nt in range(1, NT):
            for q in range(4):
                load_w_piece(nt, q)
                mul_piece(nt, q)
        # block-1 x rides behind the whole W stream (sync ring is FIFO)
        for q in range(4):
            load_x_piece(1, q)

        # ---- main: blocks of m_block m-tiles; nt-outer inside a block ----
        evac_i = 0
        for b in range(NB):
            xts = xt_t[b % 2]
            for nt in range(NT):
                # 8 rotating PSUM banks: group (nt, mb) lives on bank mb
                pms = {
                    mb: pm_pool.tile(
                        [P, n_chunk],
                        mybir.dt.float32,
                        tag=f"pm{(nt * m_block + mb) % 8}",
                        name=f"pm{(nt * m_block + mb) % 8}",
                    )
                    for mb in range(m_block)
                }
                # k-quarter-outer: each sub-group only needs its own pieces
                for q in range(4):
                    for mb in range(m_block):
                        for k in range(KQ):
                            kt = q * KQ + k
                            nc.tensor.matmul(
                                pms[mb][:],
                                xts[q][:, k, bass.ts(mb, P)],
                                wm_t[nt][q][:, k, :],
                                start=(kt == 0),
                                stop=(kt == KT - 1),
                            )
                        if q == 3:
                            # evacuate as soon as this group closes; alternate
                            # engines so no single queue bottlenecks
                            mt = b * m_block + mb
                            yo = yo_pool.tile([P, n_chunk], mybir.dt.float32, tag="yo")
                            if evac_i % 2 == 0:
                                nc.scalar.copy(yo[:], pms[mb][:])
                            else:
                                nc.vector.tensor_copy(yo[:], pms[mb][:])
                            dma_eng = nc.scalar if evac_i % 2 == 0 else nc.gpsimd
                            dma_eng.dma_start(
                                out=y[mt * P : (mt + 1) * P, bass.ts(nt, n_chunk)],
                                in_=yo[:],
                            )
                            evac_i += 1

    nc.compile()
    return nc


def _prep_host(input_, weight, mask, n_chunk=512):
    import ml_dtypes

    in_dim, out_dim = weight.shape[1], weight.shape[0]
    nt = out_dim // n_chunk
    # weight.T -> [NT, IN, n_chunk] bf16, each panel contiguous
    wtp = np.ascontiguousarray(
        weight.T.reshape(in_dim, nt, n_chunk).transpose(1, 0, 2)
    ).astype(ml_dtypes.bfloat16)
    mkp = np.ascontiguousarray(
        mask.T.reshape(in_dim, nt, n_chunk).transpose(1, 0, 2)
    ).astype(np.int8)
    rows = input_.shape[0] // N_CORES
    xbf = input_.astype(ml_dtypes.bfloat16)
    in_maps = []
    for c in range(N_CORES):
        xp = np.ascontiguousarray(xbf[c * rows : (c + 1) * rows].T)
        in_maps.append({"x": xp, "wt": wtp, "mk": mkp})
    return in_maps


_CACHE = {}


def _run(input_, weight, mask, trace=False, **build_kw):
    rows_total, in_dim = input_.shape
    out_dim = weight.shape[0]
    key = (rows_total, in_dim, out_dim, tuple(sorted(build_kw.items())))
    if key not in _CACHE:
        _CACHE[key] = build_nc(
            rows=rows_total // N_CORES, in_dim=in_dim, out_dim=out_dim, **build_kw
        )
    nc = _CACHE[key]
    in_maps = _prep_host(input_, weight, mask, build_kw.get("n_chunk", 512))
    res = run_bass_kernel_spmd(nc, in_maps, core_ids=list(range(N_CORES)), trace=trace)
    out = np.concatenate([res.results[c]["y"] for c in range(N_CORES)], axis=0)
    return out, res


def kernel(input_, weight, mask):
    input_ = np.asarray(input_, dtype=np.float32)
    weight = np.asarray(weight, dtype=np.float32)
    mask = np.asarray(mask)
    out, _ = _run(input_, weight, mask, trace=False)
    return out
